# revision 74
# baseline (speedup 1.0000x reference)
"""GNN message-passing kernel for Trainium2, 8 NeuronCores (SPMD + collectives).

Sharding: nodes by contiguous range (6250/core = 66 windows of 96); edges by
dst owner, sorted (core, src-half, window), runs padded to x128.  Per layer:
    m_pre = h[dst]@W1a + h[src]@W1b + ea@(We@W1c) + const
    aggr  = segsum(relu(m_pre)) ; the @mW2 is folded into the update weights.

Layer 0 is input-derived, so its whole message pre-activation is ONE K=81
matmul per 128-edge tile against host-shipped [x_dst; x_src; ea; 1] rows
(R81 folds Wn@W1a0, Wn@W1b0, Weff0 and all constants) - no AllGather and no
gather in layer 0.  Layers 1-2: h[dst]@W1a + ea@Weff is one matmul per tile
(host-precomputed lhsT: 96 one-hot rows + 16 eaT rows, streamed per chunk);
h[src]@W1b is fetched from the AllGathered bf16 B-table with PREPARED
dma_gather (gen_mode=1): descriptor gen on GpSimd is decoupled from the
transfer via trigger_dma, the b_full RAW dep is moved prep->trigger, and
consumers gate on 8 rotating DMA-completion sems (tile's own DMASW lane sems
are pre-bumped at prep time and do NOT gate on data).  Aggregation is a matmul
with the host-precomputed one-hot S tile accumulating into a per-window PSUM.
Readout pooling is transpose+matmul against a batch one-hot, then AllReduce.
"""
import numpy as np
import ml_dtypes

import concourse.bass as bass
import concourse.bacc as bacc
import concourse.mybir as mybir
import concourse.tile as tile
from concourse.instruction_name_ordered_set import InstructionNameOrderedSet

BF16 = ml_dtypes.bfloat16

CFG = dict(
    N=50000, E=400000, B=8, ND=32, ED=16, H=128, L=3, NC=8,
    WSEG=96, LOCORES=5, GTILES=40, QUAD=4,
)


def _derive(cfg):
    d = dict(cfg)
    d["NPC"] = d["N"] // d["NC"]
    d["NW"] = -(-d["NPC"] // d["WSEG"])
    d["SLOT"] = d["NW"] * d["WSEG"]
    d["NSTAR"] = d["NC"] * d["SLOT"]
    d["LO"] = d["LOCORES"] * d["SLOT"]
    assert d["LO"] < 32768 and d["NSTAR"] - d["LO"] < 32768
    assert d["N"] % d["NC"] == 0
    return d


def prep_host(inputs, cfg):
    c = _derive(cfg)
    N, E, B, ND, ED, H, L, NC = (c[k] for k in "N E B ND ED H L NC".split())
    NPC, NW, WSEG, SLOT, LO = c["NPC"], c["NW"], c["WSEG"], c["SLOT"], c["LO"]

    src = np.asarray(inputs["edge_index"][0], np.int64)
    dst = np.asarray(inputs["edge_index"][1], np.int64)
    batch = np.asarray(inputs["batch"], np.int64)
    x = np.asarray(inputs["x"], np.float32)
    ea = np.asarray(inputs["edge_attr"], np.float32)

    core = dst // NPC
    dloc = dst - core * NPC
    w = dloc // WSEG
    s = dloc - w * WSEG
    srow = (src // NPC) * SLOT + (src - (src // NPC) * NPC)
    sweep = (srow >= LO).astype(np.int64)

    order = np.lexsort((w, sweep, core))
    core_o, sweep_o, w_o, s_o, srow_o = (a[order] for a in (core, sweep, w, s, srow))
    eid_o = order

    grp = (core_o * 2 + sweep_o) * NW + w_o
    counts = np.bincount(grp, minlength=NC * 2 * NW).reshape(NC, 2, NW)
    TL = np.maximum(1, -(-counts[:, 0, :].max(axis=0) // 128)).astype(int)
    TH = (-(-counts[:, 1, :].max(axis=0) // 128)).astype(int)
    TLtot, THtot = int(TL.sum()), int(TH.sum())
    TT = TLtot + THtot
    offL = np.concatenate([[0], np.cumsum(TL)[:-1]]).astype(np.int64)
    offH = np.concatenate([[0], np.cumsum(TH)[:-1]]).astype(np.int64) + TLtot

    first = np.zeros(NC * 2 * NW, np.int64)
    csum = np.cumsum(np.bincount(grp, minlength=NC * 2 * NW))
    first[1:] = csum[:-1]
    rank = np.arange(E) - first[grp]
    runbase = np.where(sweep_o == 0, offL[w_o], offH[w_o]) * 128
    pos = runbase + rank

    NP = TT * 128
    SRCI = np.zeros((NC, NP), np.int16)
    DCT = np.full((NC, NP), -1.0, np.float32)
    EAP = np.zeros((NC, NP, ED), np.float32)
    idx_lin = np.where(sweep_o == 0, srow_o, srow_o - LO)
    SRCI[core_o, pos] = idx_lin.astype(np.int16)
    DCT[core_o, pos] = s_o.astype(np.float32)
    EAP[core_o, pos] = ea[eid_o]

    def wrap(a):
        return np.ascontiguousarray(np.tile(a.reshape(-1, 16).T, (8, 1)).astype(np.int16))
    idxL = np.stack([wrap(SRCI[cc, :TLtot * 128]) for cc in range(NC)])
    idxH = (np.stack([wrap(SRCI[cc, TLtot * 128:]) for cc in range(NC)])
            if THtot > 0 else np.zeros((NC, 128, 8), np.int16))

    # Host-precomputed message lhsT (one-hot dst rows 0:96 + eaT rows 96:112)
    # and aggregation one-hot rhs (ST[p, t*WSEG+s] = dst slot s at tile t row p).
    CMB = np.zeros((NC, 112, NP), BF16)
    CMB[core_o, s_o, pos] = 1.0
    CMB[:, WSEG:WSEG + ED, :] = EAP.transpose(0, 2, 1).astype(BF16)
    STH = np.zeros((NC, 128, TT * WSEG), BF16)
    t_idx, p_idx = pos // 128, pos % 128
    STH[core_o, p_idx, t_idx * WSEG + s_o] = 1.0
    # Layer-0 per-edge input rows (pure input indexing): x[dst], x[src], ea,
    # and a 1.0 marker row (carries all layer-0 constants). One K=2*ND+ED+1
    # matmul against R81 computes the full layer-0 message pre-activation.
    XPE = np.zeros((NC, 2 * ND + ED + 1, NP), BF16)
    XPE[core_o, :ND, pos] = x[dst[eid_o]].astype(BF16)
    XPE[core_o, ND:2 * ND, pos] = x[src[eid_o]].astype(BF16)
    XPE[:, 2 * ND:2 * ND + ED, :] = EAP.transpose(0, 2, 1).astype(BF16)
    XPE[core_o, 2 * ND + ED, pos] = 1.0

    xT = np.zeros((NC, ND, SLOT), np.float32)
    deg = np.zeros((NC, 1, SLOT), np.float32)
    dcnt = np.bincount(dst, minlength=N).astype(np.float32)
    NWS = -(-SLOT // 128)
    BSEL = np.zeros((NC, 128, NWS * B), BF16)
    for cc in range(NC):
        xT[cc, :, :NPC] = x[cc * NPC:(cc + 1) * NPC].T
        deg[cc, 0, :NPC] = dcnt[cc * NPC:(cc + 1) * NPC]
        bl = batch[cc * NPC:(cc + 1) * NPC].astype(np.int64)
        sl = np.arange(NPC)
        BSEL[cc, sl % 128, (sl // 128) * B + bl] = 1.0
    xT, degT = xT.astype(BF16), deg.astype(BF16)

    gcnt = np.bincount(batch, minlength=B).astype(np.float32)
    invc = (1.0 / np.maximum(gcnt, 1.0)).astype(np.float32)[None, :]

    f = lambda a: np.asarray(a, np.float32)
    Wn, bn = f(inputs["Wn"]), f(inputs["bn"])
    We, be = f(inputs["We"]), f(inputs["be"])
    Wg, bg = f(inputs["Wg"]), f(inputs["bg"])
    mW1, mb1 = f(inputs["mW1"]), f(inputs["mb1"])
    mW2, mb2 = f(inputs["mW2"]), f(inputs["mb2"])
    uW1, ub1 = f(inputs["uW1"]), f(inputs["ub1"])
    uW2, ub2 = f(inputs["uW2"]), f(inputs["ub2"])
    rW1, rb1 = f(inputs["rW1"]), f(inputs["rb1"])
    rW2, rb2 = f(inputs["rW2"]), f(inputs["rb2"])
    rW3, rb3 = f(inputs["rW3"]), f(inputs["rb3"])
    gf = f(inputs["global_feature"])

    wts = {}
    wts["emb_Wn"] = Wn.astype(BF16)
    wts["bn_col"] = np.ascontiguousarray(bn[:, None])
    # Layer-0 message: m_pre0 = x[dst]@(Wn@W1a0) + x[src]@(Wn@W1b0) + ea@Weff0
    # + (bn@W1a0 + bn@W1b0 + be@W1c0 + mb1_0), all via one K=81 matmul.
    W1a0, W1b0, W1c0 = mW1[0][:H], mW1[0][H:2 * H], mW1[0][2 * H:]
    R81 = np.concatenate([
        np.asarray(Wn @ W1a0, np.float32),
        np.asarray(Wn @ W1b0, np.float32),
        np.asarray(We @ W1c0, np.float32),
        np.asarray(bn @ W1a0 + bn @ W1b0 + be @ W1c0 + mb1[0],
                   np.float32)[None, :]], 0)
    wts["R81_0"] = R81.astype(BF16)
    for l in range(L):
        W1a, W1b, W1c = mW1[l][:H], mW1[l][H:2 * H], mW1[l][2 * H:]
        wts[f"W1a_{l}"] = W1a.astype(BF16)
        wts[f"W1b_{l}"] = W1b.astype(BF16)
        wts[f"Weff_{l}"] = (We @ W1c).astype(BF16)
        wts[f"cst_{l}"] = np.ascontiguousarray((be @ W1c + mb1[l])[None, :])
        wts[f"uW1h_{l}"] = uW1[l][:H].astype(BF16)
        wts[f"uW1a_{l}"] = (mW2[l] @ uW1[l][H:]).astype(BF16)
        wts[f"vec_{l}"] = np.ascontiguousarray((mb2[l] @ uW1[l][H:])[None, :]).astype(BF16)
        wts[f"ub1_{l}"] = np.ascontiguousarray(ub1[l][:, None])
        wts[f"uW2_{l}"] = uW2[l].astype(BF16)
        wts[f"ub2_{l}"] = np.ascontiguousarray(ub2[l][:, None])
    wts["rW1p"] = rW1[:H].astype(BF16)
    wts["rW1g"] = rW1[H:].astype(BF16)
    wts["rb1_col"] = np.ascontiguousarray(rb1[:, None])
    wts["rW2"] = rW2.astype(BF16)
    wts["rb2_col"] = np.ascontiguousarray(rb2[:, None])
    wts["rW3"] = rW3.astype(BF16)
    wts["rb3_col"] = np.ascontiguousarray(rb3[:, None])
    wts["Wg_row"] = Wg.astype(BF16)
    wts["bg_col"] = np.ascontiguousarray(bg[:, None])
    wts["gfT"] = np.ascontiguousarray(gf.T).astype(BF16)
    wts["invc"] = invc
    wts["ident"] = np.eye(128, dtype=np.float32)

    meta = dict(TL=[int(t) for t in TL], TH=[int(t) for t in TH],
                TLtot=TLtot, THtot=THtot, TT=TT, derived=c)
    percore = dict(idxL=idxL, idxH=idxH, CMB=CMB, STH=STH, XPE=XPE,
                   xT=xT, degT=degT, BSEL=BSEL)
    return meta, percore, wts


def _bcast_ap(dram_tensor, lo, hi, parts):
    """Manual AP: DRAM [1, n] slice replicated across `parts` partitions."""
    ap = dram_tensor[0:1, lo:hi]
    return bass.AP(ap.tensor, ap.offset, [[0, parts], [1, hi - lo]])


def build_bass(meta, wts_np, use_sliced_hi=True):
    import os as _os
    KPHASE = int(_os.environ.get("KPHASE", "4"))
    c = meta["derived"]
    B, ND, ED, H, L, NC = (c[k] for k in "B ND ED H L NC".split())
    NW, WSEG, SLOT, NSTAR, LO = c["NW"], c["WSEG"], c["SLOT"], c["NSTAR"], c["LO"]
    GT, QUAD = c["GTILES"], c["QUAD"]
    TL, TH, TLtot, THtot, TT = (meta[k] for k in ("TL", "TH", "TLtot", "THtot", "TT"))
    HI = NSTAR - LO
    MAXT = max(max(TL), max(TH) if TH else 0)
    f32, bf16, i16 = mybir.dt.float32, mybir.dt.bfloat16, mybir.dt.int16
    RELU = mybir.ActivationFunctionType.Relu
    IDENT = mybir.ActivationFunctionType.Identity
    ADD, MULT = mybir.AluOpType.add, mybir.AluOpType.mult
    EQ = mybir.AluOpType.is_equal

    nc = bacc.Bacc("TRN2", target_bir_lowering=False, debug=False, num_devices=NC,
                   num_swdge_queues=2)

    NP = TT * 128
    t_idxL = nc.dram_tensor("idxL", [128, max(TLtot, 1) * 8], i16, kind="ExternalInput")
    t_idxH = nc.dram_tensor("idxH", [128, max(THtot, 1) * 8], i16, kind="ExternalInput")
    t_cmb = nc.dram_tensor("CMB", [112, NP], bf16, kind="ExternalInput")
    KXL = 2 * ND + ED + 1
    t_xpe = nc.dram_tensor("XPE", [KXL, NP], bf16, kind="ExternalInput")
    t_st = nc.dram_tensor("STH", [128, TT * WSEG], bf16, kind="ExternalInput")
    t_xT = nc.dram_tensor("xT", [ND, SLOT], bf16, kind="ExternalInput")
    t_degT = nc.dram_tensor("degT", [1, SLOT], bf16, kind="ExternalInput")
    NWS = -(-SLOT // 128)
    t_bsel = nc.dram_tensor("BSEL", [128, NWS * B], bf16, kind="ExternalInput")
    wt = {k: nc.dram_tensor(k, list(v.shape),
                            bf16 if v.dtype == BF16 else f32, kind="ExternalInput")
          for k, v in wts_np.items()}
    t_out = nc.dram_tensor("out", [1, B], f32, kind="ExternalOutput")

    offL = [int(v) for v in np.concatenate([[0], np.cumsum(TL)[:-1]])]
    offH = [int(v) for v in (np.concatenate([[0], np.cumsum(TH)[:-1]]) + TLtot)]

    # Prepared (gen_mode=1) SWDGE gathers: tile's auto consumer-waits gate on
    # descriptor-prep, not DMA completion, so data consumption must be gated
    # manually on the DMA-completion sem baked into the descriptors (sem=).
    # 8 rotating sems bound the tolerated cross-ring completion skew.
    gsems = [nc.alloc_semaphore(f"gsem{i}") for i in range(8)]
    gseq = [0]

    def next_gather_gate():
        k = gseq[0]
        gseq[0] += 1
        return gsems[k % 8], 16 * (k // 8 + 1)

    with tile.TileContext(nc) as tc:
        with tc.tile_pool(name="const", bufs=1) as cpool, \
             tc.tile_pool(name="data", bufs=1) as dpool, \
             tc.tile_pool(name="comb", bufs=2) as combp, \
             tc.tile_pool(name="sw", bufs=2) as swp, \
             tc.tile_pool(name="bg", bufs=5) as bgp, \
             tc.tile_pool(name="wk", bufs=2) as wkp, \
             tc.tile_pool(name="stp", bufs=3) as stp, \
             tc.tile_pool(name="ps", bufs=2, space="PSUM") as psp, \
             tc.tile_pool(name="dram", bufs=2, space="DRAM") as drp, \
             tc.tile_pool(name="dram1", bufs=1, space="DRAM") as drp1:

            w_sb = {}
            for k, v in wts_np.items():
                tl = cpool.tile(list(v.shape), bf16 if v.dtype == BF16 else f32, tag=k)
                nc.sync.dma_start(out=tl[:], in_=wt[k][:])
                w_sb[k] = tl
            invc_rep = cpool.tile([128, B], f32, tag="invc_rep")
            nc.sync.dma_start(out=invc_rep[:], in_=_bcast_ap(wt["invc"], 0, B, 128))
            cst_rep = {}
            for l in range(L):
                t = cpool.tile([128, H], f32, tag=f"cst_rep{l}")
                nc.sync.dma_start(out=t[:], in_=_bcast_ap(wt[f"cst_{l}"], 0, H, 128))
                cst_rep[l] = t

            hT = dpool.tile([128, SLOT], f32, tag="hT")
            hTb = dpool.tile([128, SLOT], bf16, tag="hTb")
            aggr = dpool.tile([128, SLOT], bf16, tag="aggr")
            AW = dpool.tile([128, NW * 128], bf16, tag="AW")
            xT_sb = dpool.tile([ND, SLOT], bf16, tag="xT")
            nc.sync.dma_start(out=xT_sb[:], in_=t_xT[:])
            degT_sb = dpool.tile([1, SLOT], bf16, tag="degT")
            nc.sync.dma_start(out=degT_sb[:], in_=t_degT[:])
            bsel_sb = dpool.tile([128, NWS * B], bf16, tag="bsel")
            nc.sync.dma_start(out=bsel_sb[:], in_=t_bsel[:])
            idxL_sb = dpool.tile([128, max(TLtot, 1) * 8], i16, tag="idxL")
            nc.sync.dma_start(out=idxL_sb[:], in_=t_idxL[:])
            idxH_sb = dpool.tile([128, max(THtot, 1) * 8], i16, tag="idxH")
            if THtot > 0:
                nc.sync.dma_start(out=idxH_sb[:], in_=t_idxH[:])

            def strips512():
                o = 0
                while o < SLOT:
                    fz = min(512, SLOT - o)
                    yield o, fz
                    o += fz

            # embedding
            for o, fz in strips512():
                ps = psp.tile([128, 512], f32, space="PSUM", tag="pu")
                nc.tensor.matmul(out=ps[:, :fz], lhsT=w_sb["emb_Wn"][:],
                                 rhs=xT_sb[:, o:o + fz], start=True, stop=True)
                nc.scalar.activation(out=hT[:, o:o + fz], in_=ps[:, :fz],
                                     func=IDENT, bias=w_sb["bn_col"][:])
                nc.vector.tensor_copy(out=hTb[:, o:o + fz], in_=hT[:, o:o + fz])

            def build_aw_window(w, lw):
                """AW[w] = h_window @ W1a_lw (+ Weff_lw rows) from current hTb."""
                ps = psp.tile([128, 512], f32, space="PSUM", tag="pq",
                              name="psaw")[:, 0:128]
                nc.tensor.matmul(out=ps[0:WSEG, :],
                                 lhsT=hTb[:, w * WSEG:(w + 1) * WSEG],
                                 rhs=w_sb[f"W1a_{lw}"][:], start=True, stop=True)
                nc.scalar.activation(out=AW[0:WSEG, w * 128:w * 128 + 128],
                                     in_=ps[0:WSEG, :], func=IDENT)
                nc.vector.tensor_copy(out=AW[WSEG:WSEG + ED, w * 128:w * 128 + 128],
                                      in_=w_sb[f"Weff_{lw}"][:])

            def build_b_range(b_dst, so, fz, lw):
                """b_dst[so:so+fz] = hTb[:, so:so+fz]^T @ W1b_lw + cst_lw."""
                sub = 0
                while sub < fz:
                    nb = min(4, (fz - sub) // 128)
                    if nb >= 1:
                        bstr4 = stp.tile([128, 4, H], bf16, tag="bstr4")
                        for i in range(nb):
                            s0 = so + sub + i * 128
                            ps = psp.tile([128, 512], f32, space="PSUM",
                                          tag="pq", name="psb")[:, 0:128]
                            nc.tensor.matmul(out=ps[:], lhsT=hTb[:, s0:s0 + 128],
                                             rhs=w_sb[f"W1b_{lw}"][:],
                                             start=True, stop=True)
                            nc.vector.tensor_tensor(out=bstr4[:, i, :], in0=ps[:],
                                                    in1=cst_rep[lw][:], op=ADD)
                        nc.sync.dma_start(
                            out=b_dst[so + sub:so + sub + nb * 128, :].rearrange(
                                "(a p) h -> p a h", p=128),
                            in_=bstr4[:, 0:nb, :])
                        sub += nb * 128
                    else:
                        sz = fz - sub
                        ps = psp.tile([128, 512], f32, space="PSUM",
                                      tag="pq", name="psb")[:, 0:128]
                        nc.tensor.matmul(out=ps[0:sz, :],
                                         lhsT=hTb[:, so + sub:so + sub + sz],
                                         rhs=w_sb[f"W1b_{lw}"][:],
                                         start=True, stop=True)
                        bstr = stp.tile([128, H], bf16, tag="bstr")
                        nc.vector.tensor_tensor(out=bstr[0:sz, :], in0=ps[0:sz, :],
                                                in1=cst_rep[lw][0:sz, :], op=ADD)
                        nc.sync.dma_start(
                            out=b_dst[so + sub:so + sub + sz, :],
                            in_=bstr[0:sz, :])
                        sub = fz

            # Layer l+1's B table and AW windows are built inside layer l's
            # update loop, strip by strip, so the next AllGather fires right
            # after the last update strip.
            b_own_next = None
            if L > 1 and KPHASE >= 2:
                b_own_next = drp.tile([SLOT, H], bf16, tag="b_own")
            pool_ps = psp.tile([128, B], f32, space="PSUM", tag="pps",
                               name="pool_ps", bufs=1)
            nstrips = -(-SLOT // 128)

            for l in range(L if KPHASE >= 2 else 0):
                # Layer l's B table (stored during layer l-1's update) +
                # AllGather. Layer 0 needs neither: its message uses
                # host-shipped x[src]/x[dst] rows.
                ag_name, b_lo_ap, b_hi_ap = None, None, None
                b_own_cur = b_own_next
                if l + 1 < L:
                    b_own_next = drp.tile([SLOT, H], bf16, tag="b_own")
                if l > 0:
                    b_full = drp.tile([NSTAR, H], bf16, tag="b_full",
                                      addr_space="Shared")
                    ag_inst = nc.gpsimd.collective_compute(
                        "AllGather", mybir.AluOpType.bypass,
                        replica_groups=[list(range(NC))],
                        ins=[b_own_cur.opt()], outs=[b_full.opt()])
                    ag_name = ag_inst.ins.name
                    if use_sliced_hi:
                        b_lo_ap, b_hi_ap = b_full[0:LO, :], b_full[LO:NSTAR, :]
                    else:
                        b_hi = drp.tile([HI, H], bf16, tag="b_hi")
                        nc.sync.dma_start(out=b_hi[:], in_=b_full[LO:NSTAR, :])
                        b_lo_ap, b_hi_ap = b_full[0:LO, :], b_hi[:]

                def sweep(sweep_id, Tarr, offarr, tstart, idx_sb, tab_ap):
                    Ttot_s = int(sum(Tarr))
                    if Ttot_s == 0:
                        return
                    # chunk whole windows so each chunk has <= GT tiles
                    chunks, cur_w, cur_ct = [], [], 0
                    for w in range(NW):
                        Tw = Tarr[w]
                        if Tw == 0:
                            continue
                        if cur_ct > 0 and cur_ct + Tw > GT:
                            chunks.append((cur_w, cur_ct))
                            cur_w, cur_ct = [], 0
                        cur_w.append(w)
                        cur_ct += Tw
                    if cur_ct:
                        chunks.append((cur_w, cur_ct))
                    t0 = 0
                    for ws, ct in chunks:
                        np0 = (tstart + t0) * 128
                        st0 = (tstart + t0) * WSEG
                        stt = swp.tile([128, GT * WSEG], bf16, tag="stt")
                        nc.scalar.dma_start(out=stt[:, 0:ct * WSEG],
                                            in_=t_st[:, st0:st0 + ct * WSEG])
                        if l == 0:
                            xpe = combp.tile([KXL, GT * 128], bf16, tag="cmb")
                            nc.sync.dma_start(out=xpe[:, 0:ct * 128],
                                              in_=t_xpe[:, np0:np0 + ct * 128])
                        else:
                            cmb = combp.tile([112, GT * 128], bf16, tag="cmb")
                            nc.sync.dma_start(out=cmb[:, 0:ct * 128],
                                              in_=t_cmb[:, np0:np0 + ct * 128])
                            bgt = bgp.tile([128, GT, 128], bf16, tag="bgt")
                            sem, val = next_gather_gate()
                            qn_ = gseq[0] % 2
                            prep = nc.gpsimd.dma_gather(
                                out_ap=bgt[:, 0:ct, :], in_ap=tab_ap,
                                idxs_ap=idx_sb[:, t0 * 8:(t0 + ct) * 8],
                                num_idxs=ct * 128, num_idxs_reg=ct * 128,
                                elem_size=H, single_packet=False,
                                prepare_only=True, sem=sem, queue_num=qn_)
                            trig = nc.gpsimd.trigger_dma(count=None,
                                                         queue_num=qn_)
                            # Descriptor gen only reads the idx table; move
                            # the b_full (AllGather) RAW dep from the prep to
                            # the trigger so gen isn't gated on the collective.
                            if prep.ins.try_remove_dependency(ag_name):
                                ds = InstructionNameOrderedSet()
                                ds.add(ag_name)
                                trig.ins.add_sync_dependencies_from(ds)
                        tl_map = [(w, k) for w in ws for k in range(Tarr[w])]
                        t = 0
                        pag_of = {}
                        first_ms = True
                        while t < ct:
                            qn = min(QUAD, ct - t)
                            psq = psp.tile([128, QUAD * 128], f32, space="PSUM",
                                           tag="pq")
                            for j in range(qn):
                                w, _ = tl_map[t + j]
                                if l == 0:
                                    nc.tensor.matmul(
                                        out=psq[:, j * 128:(j + 1) * 128],
                                        lhsT=xpe[:, (t + j) * 128:(t + j + 1) * 128],
                                        rhs=w_sb["R81_0"][:],
                                        start=True, stop=True)
                                else:
                                    nc.tensor.matmul(
                                        out=psq[:, j * 128:(j + 1) * 128],
                                        lhsT=cmb[:, (t + j) * 128:(t + j + 1) * 128],
                                        rhs=AW[0:WSEG + ED, w * 128:(w + 1) * 128],
                                        start=True, stop=True)
                            rs = wkp.tile([128, QUAD * 128], bf16, tag="rs")
                            if l == 0:
                                nc.scalar.activation(out=rs[:, 0:qn * 128],
                                                     in_=psq[:, 0:qn * 128],
                                                     func=RELU)
                            else:
                                ms = wkp.tile([128, QUAD * 128], bf16, tag="ms")
                                ms_inst = nc.vector.tensor_tensor(
                                    out=ms[:, 0:qn * 128], in0=psq[:, 0:qn * 128],
                                    in1=bgt[:, t:t + qn, :].rearrange(
                                        "p a b -> p (a b)"),
                                    op=ADD)
                                if first_ms:
                                    ms_inst._wait_ge(sem, val)
                                    # Manual sem gates are invisible to the
                                    # scheduler: pin this consumer after the
                                    # AllGather so engine queues can't order
                                    # it ahead of the collective's producers.
                                    ds2 = InstructionNameOrderedSet()
                                    ds2.add(ag_name)
                                    ms_inst.ins.add_sync_dependencies_from(ds2)
                                    first_ms = False
                                nc.scalar.activation(out=rs[:, 0:qn * 128],
                                                     in_=ms[:, 0:qn * 128],
                                                     func=RELU)
                            for j in range(qn):
                                w, ltw = tl_map[t + j]
                                pagt = pag_of.get(w)
                                if pagt is None:
                                    pagt = psp.tile([128, WSEG], f32,
                                                    space="PSUM", tag="pagt")
                                    pag_of[w] = pagt
                                nc.tensor.matmul(
                                    out=pagt[:], lhsT=rs[:, j * 128:(j + 1) * 128],
                                    rhs=stt[:, (t + j) * WSEG:(t + j + 1) * WSEG],
                                    start=(ltw == 0), stop=(ltw == Tarr[w] - 1))
                                if ltw == Tarr[w] - 1:
                                    if sweep_id == 0:
                                        nc.scalar.activation(
                                            out=aggr[:, w * WSEG:(w + 1) * WSEG],
                                            in_=pagt[:], func=IDENT)
                                    else:
                                        nc.vector.tensor_tensor(
                                            out=aggr[:, w * WSEG:(w + 1) * WSEG],
                                            in0=aggr[:, w * WSEG:(w + 1) * WSEG],
                                            in1=pagt[:], op=ADD)
                                    del pag_of[w]
                            t += qn
                        t0 += ct

                if KPHASE >= 3:
                    sweep(0, TL, offL, 0, idxL_sb, b_lo_ap)
                    sweep(1, TH, offH, TLtot, idxH_sb, b_hi_ap)

                # update MLP (mW2 folded into uW1a; deg term via K=1 matmul).
                # For l < L-1, each freshly updated strip immediately feeds the
                # next layer's B-table strips and AW windows.
                next_aw = 0
                for o, fz in (strips512() if KPHASE >= 4 else []):
                    ps = psp.tile([128, 512], f32, space="PSUM", tag="pu")
                    nc.tensor.matmul(out=ps[:, :fz], lhsT=w_sb[f"uW1h_{l}"][:],
                                     rhs=hTb[:, o:o + fz], start=True, stop=False)
                    nc.tensor.matmul(out=ps[:, :fz], lhsT=w_sb[f"uW1a_{l}"][:],
                                     rhs=aggr[:, o:o + fz], start=False, stop=False)
                    nc.tensor.matmul(out=ps[:, :fz], lhsT=w_sb[f"vec_{l}"][:],
                                     rhs=degT_sb[:, o:o + fz], start=False, stop=True)
                    t1 = stp.tile([128, 512], bf16, tag="t1")
                    nc.scalar.activation(out=t1[:, :fz], in_=ps[:, :fz], func=RELU,
                                         bias=w_sb[f"ub1_{l}"][:])
                    ps2 = psp.tile([128, 512], f32, space="PSUM", tag="pu")
                    nc.tensor.matmul(out=ps2[:, :fz], lhsT=w_sb[f"uW2_{l}"][:],
                                     rhs=t1[:, :fz], start=True, stop=True)
                    nc.vector.scalar_tensor_tensor(
                        out=hT[:, o:o + fz], in0=ps2[:, :fz],
                        scalar=w_sb[f"ub2_{l}"][:], in1=hT[:, o:o + fz],
                        op0=ADD, op1=ADD)
                    if l < L - 1:
                        nc.vector.tensor_copy(out=hTb[:, o:o + fz], in_=hT[:, o:o + fz])
                        build_b_range(b_own_next, o, fz, l + 1)
                        while (next_aw + 1) * WSEG <= o + fz:
                            build_aw_window(next_aw, l + 1)
                            next_aw += 1
            # pooled readout: transpose h strips + matmul vs batch one-hot
            for si in range(nstrips):
                s0 = si * 128
                fz = min(128, SLOT - s0)
                pst = psp.tile([128, 512], f32, space="PSUM", tag="pq",
                               name="pstp")[:, 0:128]
                nc.tensor.transpose(out=pst[0:fz, :], in_=hT[:, s0:s0 + fz],
                                    identity=w_sb["ident"][:])
                hrow = stp.tile([128, H], bf16, tag="hrow")
                nc.scalar.activation(out=hrow[0:fz, :], in_=pst[0:fz, :],
                                     func=IDENT)
                nc.tensor.matmul(out=pool_ps[:], lhsT=hrow[0:fz, :],
                                 rhs=bsel_sb[0:fz, si * B:(si + 1) * B],
                                 start=(si == 0), stop=(si == nstrips - 1))
            pool_pt = dpool.tile([128, B], f32, tag="pool_pt")
            nc.scalar.activation(out=pool_pt[:], in_=pool_ps[:], func=IDENT)
            ari = drp1.tile([128, B], f32, tag="ari")
            aro = drp1.tile([128, B], f32, tag="aro", addr_space="Shared")
            nc.gpsimd.dma_start(out=ari[:], in_=pool_pt[:])
            nc.gpsimd.collective_compute(
                "AllReduce", ADD, replica_groups=[list(range(NC))],
                ins=[ari.opt()], outs=[aro.opt()])
            poolsum = dpool.tile([128, B], f32, tag="poolsum")
            nc.gpsimd.dma_start(out=poolsum[:], in_=aro[:])
            pooled = dpool.tile([128, B], bf16, tag="pooled")
            nc.vector.tensor_tensor(out=pooled[:], in0=poolsum[:], in1=invc_rep[:],
                                    op=MULT)
            psg = psp.tile([128, B], f32, space="PSUM", tag="pu")
            nc.tensor.matmul(out=psg[:], lhsT=w_sb["Wg_row"][:], rhs=w_sb["gfT"][:],
                             start=True, stop=True)
            g_sb = dpool.tile([128, B], bf16, tag="g_sb")
            nc.scalar.activation(out=g_sb[:], in_=psg[:], func=IDENT,
                                 bias=w_sb["bg_col"][:])
            ps1 = psp.tile([128, B], f32, space="PSUM", tag="pu")
            nc.tensor.matmul(out=ps1[:], lhsT=w_sb["rW1p"][:], rhs=pooled[:],
                             start=True, stop=False)
            nc.tensor.matmul(out=ps1[:], lhsT=w_sb["rW1g"][:], rhs=g_sb[:],
                             start=False, stop=True)
            t1r = dpool.tile([128, B], bf16, tag="t1r")
            nc.scalar.activation(out=t1r[:], in_=ps1[:], func=RELU,
                                 bias=w_sb["rb1_col"][:])
            ps2r = psp.tile([64, B], f32, space="PSUM", tag="pu")
            nc.tensor.matmul(out=ps2r[:], lhsT=w_sb["rW2"][:], rhs=t1r[:],
                             start=True, stop=True)
            t2r = dpool.tile([64, B], bf16, tag="t2r")
            nc.scalar.activation(out=t2r[:], in_=ps2r[:], func=RELU,
                                 bias=w_sb["rb2_col"][:])
            ps3 = psp.tile([1, B], f32, space="PSUM", tag="pu")
            nc.tensor.matmul(out=ps3[:], lhsT=w_sb["rW3"][:], rhs=t2r[:],
                             start=True, stop=True)
            out_sb = dpool.tile([1, B], f32, tag="out_sb")
            nc.scalar.activation(out=out_sb[:], in_=ps3[:], func=IDENT,
                                 bias=w_sb["rb3_col"][:])
            nc.sync.dma_start(out=t_out[:], in_=out_sb[:])

    nc.compile()
    return nc


def make_in_maps(meta, percore, wts, cfg):
    NC = cfg["NC"]
    in_maps = []
    for c in range(NC):
        m = {k: np.ascontiguousarray(v) for k, v in wts.items()}
        m.update(idxL=percore["idxL"][c], idxH=percore["idxH"][c],
                 CMB=percore["CMB"][c], STH=percore["STH"][c],
                 XPE=percore["XPE"][c],
                 xT=percore["xT"][c],
                 degT=percore["degT"][c], BSEL=percore["BSEL"][c])
        in_maps.append(m)
    return in_maps


def run(inputs, cfg=None, trace=False, tmpdir=None):
    cfg = cfg or CFG
    meta, percore, wts = prep_host(inputs, cfg)
    nc = build_bass(meta, wts)
    in_maps = make_in_maps(meta, percore, wts, cfg)
    from concourse.bass_utils import run_bass_kernel_spmd
    kw = {}
    if tmpdir:
        kw["tmpdir"] = tmpdir
    res = run_bass_kernel_spmd(nc, in_maps, core_ids=list(range(cfg["NC"])),
                               trace=trace, **kw)
    out = np.asarray(res.results[0]["out"], np.float32).reshape(cfg["B"], 1)
    return out, res


def kernel(**inputs) -> np.ndarray:
    out, _ = run(inputs)
    return out



# revision 83
# speedup vs baseline: 1.1509x; 1.1509x over previous
"""GNN message-passing kernel for Trainium2, 8 NeuronCores (SPMD + collectives).

Sharding: nodes by contiguous range (6250/core = 66 windows of 96); edges by
dst owner, sorted (core, src-half, window), runs padded to x128.  Per layer:
    m_pre = h[dst]@W1a + h[src]@W1b + ea@(We@W1c) + const
    aggr  = segsum(relu(m_pre)) ; the @mW2 is folded into the update weights.

Layer 0 is input-derived, so its whole message pre-activation is ONE K=81
matmul per 128-edge tile against host-shipped [x_dst; x_src; ea; 1] rows
(R81 folds Wn@W1a0, Wn@W1b0, Weff0 and all constants) - no AllGather and no
gather in layer 0.  Layers 1-2: h[dst]@W1a + ea@Weff is one matmul per tile
(host-precomputed lhsT: 96 one-hot rows + 16 eaT rows, streamed per chunk);
h[src]@W1b is fetched from the AllGathered bf16 B-table with PREPARED
dma_gather (gen_mode=1): descriptor gen on GpSimd is decoupled from the
transfer via trigger_dma, the b_full RAW dep is moved prep->trigger, and
consumers gate on 8 rotating DMA-completion sems (tile's own DMASW lane sems
are pre-bumped at prep time and do NOT gate on data).  Aggregation is a matmul
with the host-precomputed one-hot S tile accumulating into a per-window PSUM.
Readout pooling is transpose+matmul against a batch one-hot, then AllReduce.
"""
import numpy as np
import ml_dtypes

import concourse.bass as bass
import concourse.bacc as bacc
import concourse.mybir as mybir
import concourse.tile as tile
from concourse.instruction_name_ordered_set import InstructionNameOrderedSet

BF16 = ml_dtypes.bfloat16

CFG = dict(
    N=50000, E=400000, B=8, ND=32, ED=16, H=128, L=3, NC=8,
    WSEG=112, LOCORES=5, GTILES=40, QUAD=4,
)


def _derive(cfg):
    d = dict(cfg)
    d["NPC"] = d["N"] // d["NC"]
    d["NW"] = -(-d["NPC"] // d["WSEG"])
    d["SLOT"] = d["NW"] * d["WSEG"]
    d["NSTAR"] = d["NC"] * d["SLOT"]
    d["LO"] = d["LOCORES"] * d["SLOT"]
    assert d["LO"] < 32768 and d["NSTAR"] - d["LO"] < 32768
    assert d["N"] % d["NC"] == 0
    return d


def prep_host(inputs, cfg):
    c = _derive(cfg)
    N, E, B, ND, ED, H, L, NC = (c[k] for k in "N E B ND ED H L NC".split())
    NPC, NW, WSEG, SLOT, LO = c["NPC"], c["NW"], c["WSEG"], c["SLOT"], c["LO"]

    src = np.asarray(inputs["edge_index"][0], np.int64)
    dst = np.asarray(inputs["edge_index"][1], np.int64)
    batch = np.asarray(inputs["batch"], np.int64)
    x = np.asarray(inputs["x"], np.float32)
    ea = np.asarray(inputs["edge_attr"], np.float32)

    core = dst // NPC
    dloc = dst - core * NPC
    w = dloc // WSEG
    s = dloc - w * WSEG
    srow = (src // NPC) * SLOT + (src - (src // NPC) * NPC)
    sweep = (srow >= LO).astype(np.int64)

    order = np.lexsort((w, sweep, core))
    core_o, sweep_o, w_o, s_o, srow_o = (a[order] for a in (core, sweep, w, s, srow))
    eid_o = order

    grp = (core_o * 2 + sweep_o) * NW + w_o
    counts = np.bincount(grp, minlength=NC * 2 * NW).reshape(NC, 2, NW)
    TL = np.maximum(1, -(-counts[:, 0, :].max(axis=0) // 128)).astype(int)
    TH = (-(-counts[:, 1, :].max(axis=0) // 128)).astype(int)
    TLtot, THtot = int(TL.sum()), int(TH.sum())
    TT = TLtot + THtot
    offL = np.concatenate([[0], np.cumsum(TL)[:-1]]).astype(np.int64)
    offH = np.concatenate([[0], np.cumsum(TH)[:-1]]).astype(np.int64) + TLtot

    first = np.zeros(NC * 2 * NW, np.int64)
    csum = np.cumsum(np.bincount(grp, minlength=NC * 2 * NW))
    first[1:] = csum[:-1]
    rank = np.arange(E) - first[grp]
    runbase = np.where(sweep_o == 0, offL[w_o], offH[w_o]) * 128
    pos = runbase + rank

    NP = TT * 128
    SRCI = np.zeros((NC, NP), np.int16)
    DCT = np.full((NC, NP), -1.0, np.float32)
    EAP = np.zeros((NC, NP, ED), np.float32)
    idx_lin = np.where(sweep_o == 0, srow_o, srow_o - LO)
    SRCI[core_o, pos] = idx_lin.astype(np.int16)
    DCT[core_o, pos] = s_o.astype(np.float32)
    EAP[core_o, pos] = ea[eid_o]

    def wrap(a):
        return np.ascontiguousarray(np.tile(a.reshape(-1, 16).T, (8, 1)).astype(np.int16))
    idxL = np.stack([wrap(SRCI[cc, :TLtot * 128]) for cc in range(NC)])
    idxH = (np.stack([wrap(SRCI[cc, TLtot * 128:]) for cc in range(NC)])
            if THtot > 0 else np.zeros((NC, 128, 8), np.int16))

    # Host-precomputed message lhsT (one-hot dst rows 0:96 + eaT rows 96:112)
    # and aggregation one-hot rhs (ST[p, t*WSEG+s] = dst slot s at tile t row p).
    CMB = np.zeros((NC, WSEG + ED, NP), BF16)
    CMB[core_o, s_o, pos] = 1.0
    CMB[:, WSEG:WSEG + ED, :] = EAP.transpose(0, 2, 1).astype(BF16)
    STH = np.zeros((NC, 128, TT * WSEG), BF16)
    t_idx, p_idx = pos // 128, pos % 128
    STH[core_o, p_idx, t_idx * WSEG + s_o] = 1.0
    # Layer-0 per-edge input rows (pure input indexing): x[dst], x[src], ea,
    # and a 1.0 marker row (carries all layer-0 constants). One K=2*ND+ED+1
    # matmul against R81 computes the full layer-0 message pre-activation.
    XPE = np.zeros((NC, 2 * ND + ED + 1, NP), BF16)
    XPE[core_o, :ND, pos] = x[dst[eid_o]].astype(BF16)
    XPE[core_o, ND:2 * ND, pos] = x[src[eid_o]].astype(BF16)
    XPE[:, 2 * ND:2 * ND + ED, :] = EAP.transpose(0, 2, 1).astype(BF16)
    XPE[core_o, 2 * ND + ED, pos] = 1.0

    xT = np.zeros((NC, ND, SLOT), np.float32)
    deg = np.zeros((NC, 1, SLOT), np.float32)
    dcnt = np.bincount(dst, minlength=N).astype(np.float32)
    NWS = -(-SLOT // 128)
    BSEL = np.zeros((NC, 128, NWS * B), BF16)
    for cc in range(NC):
        xT[cc, :, :NPC] = x[cc * NPC:(cc + 1) * NPC].T
        deg[cc, 0, :NPC] = dcnt[cc * NPC:(cc + 1) * NPC]
        bl = batch[cc * NPC:(cc + 1) * NPC].astype(np.int64)
        sl = np.arange(NPC)
        BSEL[cc, sl % 128, (sl // 128) * B + bl] = 1.0
    xT, degT = xT.astype(BF16), deg.astype(BF16)

    gcnt = np.bincount(batch, minlength=B).astype(np.float32)
    invc = (1.0 / np.maximum(gcnt, 1.0)).astype(np.float32)[None, :]

    f = lambda a: np.asarray(a, np.float32)
    Wn, bn = f(inputs["Wn"]), f(inputs["bn"])
    We, be = f(inputs["We"]), f(inputs["be"])
    Wg, bg = f(inputs["Wg"]), f(inputs["bg"])
    mW1, mb1 = f(inputs["mW1"]), f(inputs["mb1"])
    mW2, mb2 = f(inputs["mW2"]), f(inputs["mb2"])
    uW1, ub1 = f(inputs["uW1"]), f(inputs["ub1"])
    uW2, ub2 = f(inputs["uW2"]), f(inputs["ub2"])
    rW1, rb1 = f(inputs["rW1"]), f(inputs["rb1"])
    rW2, rb2 = f(inputs["rW2"]), f(inputs["rb2"])
    rW3, rb3 = f(inputs["rW3"]), f(inputs["rb3"])
    gf = f(inputs["global_feature"])

    wts = {}
    wts["emb_Wn"] = Wn.astype(BF16)
    wts["bn_col"] = np.ascontiguousarray(bn[:, None])
    # Layer-0 message: m_pre0 = x[dst]@(Wn@W1a0) + x[src]@(Wn@W1b0) + ea@Weff0
    # + (bn@W1a0 + bn@W1b0 + be@W1c0 + mb1_0), all via one K=81 matmul.
    W1a0, W1b0, W1c0 = mW1[0][:H], mW1[0][H:2 * H], mW1[0][2 * H:]
    R81 = np.concatenate([
        np.asarray(Wn @ W1a0, np.float32),
        np.asarray(Wn @ W1b0, np.float32),
        np.asarray(We @ W1c0, np.float32),
        np.asarray(bn @ W1a0 + bn @ W1b0 + be @ W1c0 + mb1[0],
                   np.float32)[None, :]], 0)
    wts["R81_0"] = R81.astype(BF16)
    for l in range(L):
        W1a, W1b, W1c = mW1[l][:H], mW1[l][H:2 * H], mW1[l][2 * H:]
        wts[f"W1a_{l}"] = W1a.astype(BF16)
        wts[f"W1b_{l}"] = W1b.astype(BF16)
        wts[f"Weff_{l}"] = (We @ W1c).astype(BF16)
        wts[f"cst_{l}"] = np.ascontiguousarray((be @ W1c + mb1[l])[None, :])
        wts[f"uW1h_{l}"] = uW1[l][:H].astype(BF16)
        wts[f"uW1a_{l}"] = (mW2[l] @ uW1[l][H:]).astype(BF16)
        wts[f"vec_{l}"] = np.ascontiguousarray((mb2[l] @ uW1[l][H:])[None, :]).astype(BF16)
        wts[f"ub1_{l}"] = np.ascontiguousarray(ub1[l][:, None])
        wts[f"uW2_{l}"] = uW2[l].astype(BF16)
        wts[f"ub2_{l}"] = np.ascontiguousarray(ub2[l][:, None])
    wts["rW1p"] = rW1[:H].astype(BF16)
    wts["rW1g"] = rW1[H:].astype(BF16)
    wts["rb1_col"] = np.ascontiguousarray(rb1[:, None])
    wts["rW2"] = rW2.astype(BF16)
    wts["rb2_col"] = np.ascontiguousarray(rb2[:, None])
    wts["rW3"] = rW3.astype(BF16)
    wts["rb3_col"] = np.ascontiguousarray(rb3[:, None])
    wts["Wg_row"] = Wg.astype(BF16)
    wts["bg_col"] = np.ascontiguousarray(bg[:, None])
    wts["gfT"] = np.ascontiguousarray(gf.T).astype(BF16)
    wts["invc"] = invc
    wts["ident"] = np.eye(128, dtype=np.float32)

    meta = dict(TL=[int(t) for t in TL], TH=[int(t) for t in TH],
                TLtot=TLtot, THtot=THtot, TT=TT, derived=c)
    percore = dict(idxL=idxL, idxH=idxH, CMB=CMB, STH=STH, XPE=XPE,
                   xT=xT, degT=degT, BSEL=BSEL)
    return meta, percore, wts


def _bcast_ap(dram_tensor, lo, hi, parts):
    """Manual AP: DRAM [1, n] slice replicated across `parts` partitions."""
    ap = dram_tensor[0:1, lo:hi]
    return bass.AP(ap.tensor, ap.offset, [[0, parts], [1, hi - lo]])


def build_bass(meta, wts_np, use_sliced_hi=True):
    import os as _os
    KPHASE = int(_os.environ.get("KPHASE", "4"))
    c = meta["derived"]
    B, ND, ED, H, L, NC = (c[k] for k in "B ND ED H L NC".split())
    NW, WSEG, SLOT, NSTAR, LO = c["NW"], c["WSEG"], c["SLOT"], c["NSTAR"], c["LO"]
    GT, QUAD = c["GTILES"], c["QUAD"]
    TL, TH, TLtot, THtot, TT = (meta[k] for k in ("TL", "TH", "TLtot", "THtot", "TT"))
    HI = NSTAR - LO
    MAXT = max(max(TL), max(TH) if TH else 0)
    f32, bf16, i16 = mybir.dt.float32, mybir.dt.bfloat16, mybir.dt.int16
    RELU = mybir.ActivationFunctionType.Relu
    IDENT = mybir.ActivationFunctionType.Identity
    ADD, MULT = mybir.AluOpType.add, mybir.AluOpType.mult
    EQ = mybir.AluOpType.is_equal

    nc = bacc.Bacc("TRN2", target_bir_lowering=False, debug=False, num_devices=NC,
                   num_swdge_queues=2)

    NP = TT * 128
    t_idxL = nc.dram_tensor("idxL", [128, max(TLtot, 1) * 8], i16, kind="ExternalInput")
    t_idxH = nc.dram_tensor("idxH", [128, max(THtot, 1) * 8], i16, kind="ExternalInput")
    t_cmb = nc.dram_tensor("CMB", [WSEG + ED, NP], bf16, kind="ExternalInput")
    KXL = 2 * ND + ED + 1
    t_xpe = nc.dram_tensor("XPE", [KXL, NP], bf16, kind="ExternalInput")
    t_st = nc.dram_tensor("STH", [128, TT * WSEG], bf16, kind="ExternalInput")
    t_xT = nc.dram_tensor("xT", [ND, SLOT], bf16, kind="ExternalInput")
    t_degT = nc.dram_tensor("degT", [1, SLOT], bf16, kind="ExternalInput")
    NWS = -(-SLOT // 128)
    t_bsel = nc.dram_tensor("BSEL", [128, NWS * B], bf16, kind="ExternalInput")
    wt = {k: nc.dram_tensor(k, list(v.shape),
                            bf16 if v.dtype == BF16 else f32, kind="ExternalInput")
          for k, v in wts_np.items()}
    t_out = nc.dram_tensor("out", [1, B], f32, kind="ExternalOutput")

    offL = [int(v) for v in np.concatenate([[0], np.cumsum(TL)[:-1]])]
    offH = [int(v) for v in (np.concatenate([[0], np.cumsum(TH)[:-1]]) + TLtot)]

    # Prepared (gen_mode=1) SWDGE gathers: tile's auto consumer-waits gate on
    # descriptor-prep, not DMA completion, so data consumption must be gated
    # manually on the DMA-completion sem baked into the descriptors (sem=).
    # 8 rotating sems bound the tolerated cross-ring completion skew.
    gsems = [nc.alloc_semaphore(f"gsem{i}") for i in range(8)]
    gseq = [0]

    def next_gather_gate():
        k = gseq[0]
        gseq[0] += 1
        return gsems[k % 8], 16 * (k // 8 + 1)

    with tile.TileContext(nc) as tc:
        with tc.tile_pool(name="const", bufs=1) as cpool, \
             tc.tile_pool(name="data", bufs=1) as dpool, \
             tc.tile_pool(name="comb", bufs=2) as combp, \
             tc.tile_pool(name="sw", bufs=2) as swp, \
             tc.tile_pool(name="bg", bufs=4) as bgp, \
             tc.tile_pool(name="wk", bufs=3) as wkp, \
             tc.tile_pool(name="stp", bufs=3) as stp, \
             tc.tile_pool(name="ps", bufs=2, space="PSUM") as psp, \
             tc.tile_pool(name="dram", bufs=2, space="DRAM") as drp, \
             tc.tile_pool(name="dram1", bufs=1, space="DRAM") as drp1:

            w_sb = {}
            for k, v in wts_np.items():
                tl = cpool.tile(list(v.shape), bf16 if v.dtype == BF16 else f32, tag=k)
                nc.sync.dma_start(out=tl[:], in_=wt[k][:])
                w_sb[k] = tl
            invc_rep = cpool.tile([128, B], f32, tag="invc_rep")
            nc.sync.dma_start(out=invc_rep[:], in_=_bcast_ap(wt["invc"], 0, B, 128))
            cst_rep = {}
            for l in range(L):
                t = cpool.tile([128, H], f32, tag=f"cst_rep{l}")
                nc.sync.dma_start(out=t[:], in_=_bcast_ap(wt[f"cst_{l}"], 0, H, 128))
                cst_rep[l] = t

            hT = dpool.tile([128, SLOT], f32, tag="hT")
            hTb = dpool.tile([128, SLOT], bf16, tag="hTb")
            aggr = dpool.tile([128, SLOT], bf16, tag="aggr")
            AW = dpool.tile([128, NW * 128], bf16, tag="AW")
            degT_sb = dpool.tile([1, SLOT], bf16, tag="degT")
            nc.sync.dma_start(out=degT_sb[:], in_=t_degT[:])
            bsel_sb = dpool.tile([128, NWS * B], bf16, tag="bsel")
            nc.sync.dma_start(out=bsel_sb[:], in_=t_bsel[:])
            idxL_sb = dpool.tile([128, max(TLtot, 1) * 8], i16, tag="idxL")
            nc.sync.dma_start(out=idxL_sb[:], in_=t_idxL[:])
            idxH_sb = dpool.tile([128, max(THtot, 1) * 8], i16, tag="idxH")
            if THtot > 0:
                nc.sync.dma_start(out=idxH_sb[:], in_=t_idxH[:])

            def strips512():
                o = 0
                while o < SLOT:
                    fz = min(512, SLOT - o)
                    yield o, fz
                    o += fz

            # embedding
            xT_sb = dpool.tile([ND, SLOT], bf16, tag="xT")
            nc.sync.dma_start(out=xT_sb[:], in_=t_xT[:])
            for o, fz in strips512():
                ps = psp.tile([128, 512], f32, space="PSUM", tag="pu")
                nc.tensor.matmul(out=ps[:, :fz], lhsT=w_sb["emb_Wn"][:],
                                 rhs=xT_sb[:, o:o + fz], start=True, stop=True)
                nc.scalar.activation(out=hT[:, o:o + fz], in_=ps[:, :fz],
                                     func=IDENT, bias=w_sb["bn_col"][:])
                nc.vector.tensor_copy(out=hTb[:, o:o + fz], in_=hT[:, o:o + fz])

            def build_aw_window(w, lw):
                """AW[w] = h_window @ W1a_lw from current hTb (Weff rows are
                filled once per layer by fill_weff_rows — engine ops can't
                address base partition WSEG when it isn't 32-aligned)."""
                ps = psp.tile([128, 512], f32, space="PSUM", tag="pq",
                              name="psaw")[:, 0:128]
                nc.tensor.matmul(out=ps[0:WSEG, :],
                                 lhsT=hTb[:, w * WSEG:(w + 1) * WSEG],
                                 rhs=w_sb[f"W1a_{lw}"][:], start=True, stop=True)
                nc.scalar.activation(out=AW[0:WSEG, w * 128:w * 128 + 128],
                                     in_=ps[0:WSEG, :], func=IDENT)

            def fill_weff_rows(lw):
                """One DMA replicating Weff_lw into AW rows WSEG:WSEG+ED for
                every window."""
                ap0 = wt[f"Weff_{lw}"][:]
                rep = bass.AP(ap0.tensor, ap0.offset,
                              [[128, ED], [0, NW], [1, 128]])
                nc.sync.dma_start(
                    out=AW[WSEG:WSEG + ED, :].rearrange("p (w c) -> p w c",
                                                        c=128),
                    in_=rep)

            def build_b_range(b_dst, so, fz, lw):
                """b_dst[so:so+fz] = hTb[:, so:so+fz]^T @ W1b_lw + cst_lw."""
                sub = 0
                while sub < fz:
                    nb = min(4, (fz - sub) // 128)
                    if nb >= 1:
                        bstr4 = stp.tile([128, 4, H], bf16, tag="bstr4")
                        for i in range(nb):
                            s0 = so + sub + i * 128
                            ps = psp.tile([128, 512], f32, space="PSUM",
                                          tag="pq", name="psb")[:, 0:128]
                            nc.tensor.matmul(out=ps[:], lhsT=hTb[:, s0:s0 + 128],
                                             rhs=w_sb[f"W1b_{lw}"][:],
                                             start=True, stop=True)
                            nc.vector.tensor_tensor(out=bstr4[:, i, :], in0=ps[:],
                                                    in1=cst_rep[lw][:], op=ADD)
                        nc.sync.dma_start(
                            out=b_dst[so + sub:so + sub + nb * 128, :].rearrange(
                                "(a p) h -> p a h", p=128),
                            in_=bstr4[:, 0:nb, :])
                        sub += nb * 128
                    else:
                        sz = fz - sub
                        ps = psp.tile([128, 512], f32, space="PSUM",
                                      tag="pq", name="psb")[:, 0:128]
                        nc.tensor.matmul(out=ps[0:sz, :],
                                         lhsT=hTb[:, so + sub:so + sub + sz],
                                         rhs=w_sb[f"W1b_{lw}"][:],
                                         start=True, stop=True)
                        bstr = stp.tile([128, H], bf16, tag="bstr")
                        nc.vector.tensor_tensor(out=bstr[0:sz, :], in0=ps[0:sz, :],
                                                in1=cst_rep[lw][0:sz, :], op=ADD)
                        nc.sync.dma_start(
                            out=b_dst[so + sub:so + sub + sz, :],
                            in_=bstr[0:sz, :])
                        sub = fz

            # Layer l+1's B table and AW windows are built inside layer l's
            # update loop, strip by strip, so the next AllGather fires right
            # after the last update strip.
            b_own_next = None
            if L > 1 and KPHASE >= 2:
                b_own_next = drp.tile([SLOT, H], bf16, tag="b_own")
            pool_ps = psp.tile([128, B], f32, space="PSUM", tag="pps",
                               name="pool_ps", bufs=1)
            nstrips = -(-SLOT // 128)

            for l in range(L if KPHASE >= 2 else 0):
                # Layer l's B table (stored during layer l-1's update) +
                # AllGather. Layer 0 needs neither: its message uses
                # host-shipped x[src]/x[dst] rows.
                ag_name, b_lo_ap, b_hi_ap = None, None, None
                b_own_cur = b_own_next
                if l + 1 < L:
                    b_own_next = drp.tile([SLOT, H], bf16, tag="b_own")
                if l > 0:
                    b_full = drp.tile([NSTAR, H], bf16, tag="b_full",
                                      addr_space="Shared")
                    ag_inst = nc.gpsimd.collective_compute(
                        "AllGather", mybir.AluOpType.bypass,
                        replica_groups=[list(range(NC))],
                        ins=[b_own_cur.opt()], outs=[b_full.opt()])
                    ag_name = ag_inst.ins.name
                    if use_sliced_hi:
                        b_lo_ap, b_hi_ap = b_full[0:LO, :], b_full[LO:NSTAR, :]
                    else:
                        b_hi = drp.tile([HI, H], bf16, tag="b_hi")
                        nc.sync.dma_start(out=b_hi[:], in_=b_full[LO:NSTAR, :])
                        b_lo_ap, b_hi_ap = b_full[0:LO, :], b_hi[:]

                def sweep(sweep_id, Tarr, offarr, tstart, idx_sb, tab_ap):
                    Ttot_s = int(sum(Tarr))
                    if Ttot_s == 0:
                        return
                    # chunk whole windows so each chunk has <= GT tiles
                    chunks, cur_w, cur_ct = [], [], 0
                    for w in range(NW):
                        Tw = Tarr[w]
                        if Tw == 0:
                            continue
                        if cur_ct > 0 and cur_ct + Tw > GT:
                            chunks.append((cur_w, cur_ct))
                            cur_w, cur_ct = [], 0
                        cur_w.append(w)
                        cur_ct += Tw
                    if cur_ct:
                        chunks.append((cur_w, cur_ct))
                    t0 = 0
                    for ws, ct in chunks:
                        np0 = (tstart + t0) * 128
                        st0 = (tstart + t0) * WSEG
                        stt = swp.tile([128, GT * WSEG], bf16, tag="stt")
                        nc.scalar.dma_start(out=stt[:, 0:ct * WSEG],
                                            in_=t_st[:, st0:st0 + ct * WSEG])
                        if l == 0:
                            xpe = combp.tile([KXL, GT * 128], bf16, tag="cmb")
                            nc.sync.dma_start(out=xpe[:, 0:ct * 128],
                                              in_=t_xpe[:, np0:np0 + ct * 128])
                        else:
                            cmb = combp.tile([WSEG + ED, GT * 128], bf16, tag="cmb")
                            nc.sync.dma_start(out=cmb[:, 0:ct * 128],
                                              in_=t_cmb[:, np0:np0 + ct * 128])
                            bgt = bgp.tile([128, GT, 128], bf16, tag="bgt")
                            sem, val = next_gather_gate()
                            qn_ = gseq[0] % 2
                            prep = nc.gpsimd.dma_gather(
                                out_ap=bgt[:, 0:ct, :], in_ap=tab_ap,
                                idxs_ap=idx_sb[:, t0 * 8:(t0 + ct) * 8],
                                num_idxs=ct * 128, num_idxs_reg=ct * 128,
                                elem_size=H, single_packet=False,
                                prepare_only=True, sem=sem, queue_num=qn_)
                            trig = nc.gpsimd.trigger_dma(count=None,
                                                         queue_num=qn_)
                            # Descriptor gen only reads the idx table; move
                            # the b_full (AllGather) RAW dep from the prep to
                            # the trigger so gen isn't gated on the collective.
                            if prep.ins.try_remove_dependency(ag_name):
                                ds = InstructionNameOrderedSet()
                                ds.add(ag_name)
                                trig.ins.add_sync_dependencies_from(ds)
                        tl_map = [(w, k) for w in ws for k in range(Tarr[w])]
                        t = 0
                        pag_of = {}
                        first_ms = True
                        while t < ct:
                            qn = min(QUAD, ct - t)
                            psq = psp.tile([128, QUAD * 128], f32, space="PSUM",
                                           tag="pq")
                            for j in range(qn):
                                w, _ = tl_map[t + j]
                                if l == 0:
                                    nc.tensor.matmul(
                                        out=psq[:, j * 128:(j + 1) * 128],
                                        lhsT=xpe[:, (t + j) * 128:(t + j + 1) * 128],
                                        rhs=w_sb["R81_0"][:],
                                        start=True, stop=True)
                                else:
                                    nc.tensor.matmul(
                                        out=psq[:, j * 128:(j + 1) * 128],
                                        lhsT=cmb[:, (t + j) * 128:(t + j + 1) * 128],
                                        rhs=AW[0:WSEG + ED, w * 128:(w + 1) * 128],
                                        start=True, stop=True)
                            rs = wkp.tile([128, QUAD * 128], bf16, tag="rs")
                            if l == 0:
                                nc.scalar.activation(out=rs[:, 0:qn * 128],
                                                     in_=psq[:, 0:qn * 128],
                                                     func=RELU)
                            else:
                                ms = wkp.tile([128, QUAD * 128], bf16, tag="ms")
                                ms_inst = nc.vector.tensor_tensor(
                                    out=ms[:, 0:qn * 128], in0=psq[:, 0:qn * 128],
                                    in1=bgt[:, t:t + qn, :].rearrange(
                                        "p a b -> p (a b)"),
                                    op=ADD)
                                if first_ms:
                                    ms_inst._wait_ge(sem, val)
                                    # Manual sem gates are invisible to the
                                    # scheduler: pin this consumer after the
                                    # AllGather so engine queues can't order
                                    # it ahead of the collective's producers.
                                    ds2 = InstructionNameOrderedSet()
                                    ds2.add(ag_name)
                                    ms_inst.ins.add_sync_dependencies_from(ds2)
                                    first_ms = False
                                nc.scalar.activation(out=rs[:, 0:qn * 128],
                                                     in_=ms[:, 0:qn * 128],
                                                     func=RELU)
                            for j in range(qn):
                                w, ltw = tl_map[t + j]
                                pagt = pag_of.get(w)
                                if pagt is None:
                                    pagt = psp.tile([128, WSEG], f32,
                                                    space="PSUM", tag="pagt")
                                    pag_of[w] = pagt
                                nc.tensor.matmul(
                                    out=pagt[:], lhsT=rs[:, j * 128:(j + 1) * 128],
                                    rhs=stt[:, (t + j) * WSEG:(t + j + 1) * WSEG],
                                    start=(ltw == 0), stop=(ltw == Tarr[w] - 1))
                                if ltw == Tarr[w] - 1:
                                    if sweep_id == 0:
                                        nc.scalar.activation(
                                            out=aggr[:, w * WSEG:(w + 1) * WSEG],
                                            in_=pagt[:], func=IDENT)
                                    else:
                                        nc.vector.tensor_tensor(
                                            out=aggr[:, w * WSEG:(w + 1) * WSEG],
                                            in0=aggr[:, w * WSEG:(w + 1) * WSEG],
                                            in1=pagt[:], op=ADD)
                                    del pag_of[w]
                            t += qn
                        t0 += ct

                if KPHASE >= 3:
                    sweep(0, TL, offL, 0, idxL_sb, b_lo_ap)
                    sweep(1, TH, offH, TLtot, idxH_sb, b_hi_ap)

                # update MLP (mW2 folded into uW1a; deg term via K=1 matmul).
                # For l < L-1, each freshly updated strip immediately feeds the
                # next layer's B-table strips and AW windows.
                next_aw = 0
                if l < L - 1 and KPHASE >= 4:
                    fill_weff_rows(l + 1)
                for o, fz in (strips512() if KPHASE >= 4 else []):
                    ps = psp.tile([128, 512], f32, space="PSUM", tag="pu")
                    nc.tensor.matmul(out=ps[:, :fz], lhsT=w_sb[f"uW1h_{l}"][:],
                                     rhs=hTb[:, o:o + fz], start=True, stop=False)
                    nc.tensor.matmul(out=ps[:, :fz], lhsT=w_sb[f"uW1a_{l}"][:],
                                     rhs=aggr[:, o:o + fz], start=False, stop=False)
                    nc.tensor.matmul(out=ps[:, :fz], lhsT=w_sb[f"vec_{l}"][:],
                                     rhs=degT_sb[:, o:o + fz], start=False, stop=True)
                    t1 = stp.tile([128, 512], bf16, tag="t1")
                    nc.scalar.activation(out=t1[:, :fz], in_=ps[:, :fz], func=RELU,
                                         bias=w_sb[f"ub1_{l}"][:])
                    ps2 = psp.tile([128, 512], f32, space="PSUM", tag="pu")
                    nc.tensor.matmul(out=ps2[:, :fz], lhsT=w_sb[f"uW2_{l}"][:],
                                     rhs=t1[:, :fz], start=True, stop=True)
                    nc.vector.scalar_tensor_tensor(
                        out=hT[:, o:o + fz], in0=ps2[:, :fz],
                        scalar=w_sb[f"ub2_{l}"][:], in1=hT[:, o:o + fz],
                        op0=ADD, op1=ADD)
                    if l < L - 1:
                        nc.vector.tensor_copy(out=hTb[:, o:o + fz], in_=hT[:, o:o + fz])
                        build_b_range(b_own_next, o, fz, l + 1)
                        while (next_aw + 1) * WSEG <= o + fz:
                            build_aw_window(next_aw, l + 1)
                            next_aw += 1
            # pooled readout: transpose h strips + matmul vs batch one-hot
            for si in range(nstrips):
                s0 = si * 128
                fz = min(128, SLOT - s0)
                pst = psp.tile([128, 512], f32, space="PSUM", tag="pq",
                               name="pstp")[:, 0:128]
                nc.tensor.transpose(out=pst[0:fz, :], in_=hT[:, s0:s0 + fz],
                                    identity=w_sb["ident"][:])
                hrow = stp.tile([128, H], bf16, tag="hrow")
                nc.scalar.activation(out=hrow[0:fz, :], in_=pst[0:fz, :],
                                     func=IDENT)
                nc.tensor.matmul(out=pool_ps[:], lhsT=hrow[0:fz, :],
                                 rhs=bsel_sb[0:fz, si * B:(si + 1) * B],
                                 start=(si == 0), stop=(si == nstrips - 1))
            pool_pt = dpool.tile([128, B], f32, tag="pool_pt")
            nc.scalar.activation(out=pool_pt[:], in_=pool_ps[:], func=IDENT)
            ari = drp1.tile([128, B], f32, tag="ari")
            aro = drp1.tile([128, B], f32, tag="aro", addr_space="Shared")
            nc.gpsimd.dma_start(out=ari[:], in_=pool_pt[:])
            nc.gpsimd.collective_compute(
                "AllReduce", ADD, replica_groups=[list(range(NC))],
                ins=[ari.opt()], outs=[aro.opt()])
            poolsum = dpool.tile([128, B], f32, tag="poolsum")
            nc.gpsimd.dma_start(out=poolsum[:], in_=aro[:])
            pooled = dpool.tile([128, B], bf16, tag="pooled")
            nc.vector.tensor_tensor(out=pooled[:], in0=poolsum[:], in1=invc_rep[:],
                                    op=MULT)
            psg = psp.tile([128, B], f32, space="PSUM", tag="pu")
            nc.tensor.matmul(out=psg[:], lhsT=w_sb["Wg_row"][:], rhs=w_sb["gfT"][:],
                             start=True, stop=True)
            g_sb = dpool.tile([128, B], bf16, tag="g_sb")
            nc.scalar.activation(out=g_sb[:], in_=psg[:], func=IDENT,
                                 bias=w_sb["bg_col"][:])
            ps1 = psp.tile([128, B], f32, space="PSUM", tag="pu")
            nc.tensor.matmul(out=ps1[:], lhsT=w_sb["rW1p"][:], rhs=pooled[:],
                             start=True, stop=False)
            nc.tensor.matmul(out=ps1[:], lhsT=w_sb["rW1g"][:], rhs=g_sb[:],
                             start=False, stop=True)
            t1r = dpool.tile([128, B], bf16, tag="t1r")
            nc.scalar.activation(out=t1r[:], in_=ps1[:], func=RELU,
                                 bias=w_sb["rb1_col"][:])
            ps2r = psp.tile([64, B], f32, space="PSUM", tag="pu")
            nc.tensor.matmul(out=ps2r[:], lhsT=w_sb["rW2"][:], rhs=t1r[:],
                             start=True, stop=True)
            t2r = dpool.tile([64, B], bf16, tag="t2r")
            nc.scalar.activation(out=t2r[:], in_=ps2r[:], func=RELU,
                                 bias=w_sb["rb2_col"][:])
            ps3 = psp.tile([1, B], f32, space="PSUM", tag="pu")
            nc.tensor.matmul(out=ps3[:], lhsT=w_sb["rW3"][:], rhs=t2r[:],
                             start=True, stop=True)
            out_sb = dpool.tile([1, B], f32, tag="out_sb")
            nc.scalar.activation(out=out_sb[:], in_=ps3[:], func=IDENT,
                                 bias=w_sb["rb3_col"][:])
            nc.sync.dma_start(out=t_out[:], in_=out_sb[:])

    nc.compile()
    return nc


def make_in_maps(meta, percore, wts, cfg):
    NC = cfg["NC"]
    in_maps = []
    for c in range(NC):
        m = {k: np.ascontiguousarray(v) for k, v in wts.items()}
        m.update(idxL=percore["idxL"][c], idxH=percore["idxH"][c],
                 CMB=percore["CMB"][c], STH=percore["STH"][c],
                 XPE=percore["XPE"][c],
                 xT=percore["xT"][c],
                 degT=percore["degT"][c], BSEL=percore["BSEL"][c])
        in_maps.append(m)
    return in_maps


def run(inputs, cfg=None, trace=False, tmpdir=None):
    cfg = cfg or CFG
    meta, percore, wts = prep_host(inputs, cfg)
    nc = build_bass(meta, wts)
    in_maps = make_in_maps(meta, percore, wts, cfg)
    from concourse.bass_utils import run_bass_kernel_spmd
    kw = {}
    if tmpdir:
        kw["tmpdir"] = tmpdir
    res = run_bass_kernel_spmd(nc, in_maps, core_ids=list(range(cfg["NC"])),
                               trace=trace, **kw)
    out = np.asarray(res.results[0]["out"], np.float32).reshape(cfg["B"], 1)
    return out, res


def kernel(**inputs) -> np.ndarray:
    out, _ = run(inputs)
    return out

